# revision 5
# baseline (speedup 1.0000x reference)
"""Multi-head attention (B=2, S=2048, E=1024, H=16, D=64) on 8 Trainium2 NeuronCores.

Sharding (tensor-parallel over heads x data-parallel over batch):
core c -> batch b=c//4, head-group g=c%4 (4 heads per core). Each core runs the
full per-group attention pipeline; the host sums the 4 partial out-projections
per batch element and adds b_out.

Per-core pipeline (all matmul streams bf16, PSUM accumulation fp32):
  S1  qkT[f, s]  = (W_qk_aug.T @ xT_aug)   f in [q(256)|k(256)], W_q prescaled
                                           by 1/sqrt(D); bias via ones-row MM
  S1b v[s, f]    = (xT.T @ Wv_aug)         Wv has a unit column per head so each
                                           [128,4,65] v-tile carries a ones col
  A   sT[k, q]   = kT_h @ qT_h             2 heads packed concurrently via 64x128
                                           row tiling (tile_position (0,0)/(64,0))
  exp es = exp(sT)                         split between ACT (true exp, bf16 out)
                                           and DVE (Schraudolph int16 exp2 trick
                                           written through a bitcast-to-int16 AP)
  B   poT[65, q] = [v_h|1].T @ es-half     accumulated over k-tiles; row 64 is the
                                           softmax denominator
  N   aT = poT[0:64] * recip(denom)        denom broadcast across partitions via
                                           a DRAM bounce
  S3  y[s, e]    = aT-chunks.T @ Wo        partial out-projection (b_out on host)
"""

from contextlib import nullcontext

import numpy as np
import ml_dtypes
import concourse.bass as bass
import concourse.tile as tile
from concourse import bacc, mybir
from concourse.bass_utils import run_bass_kernel_spmd

F32 = mybir.dt.float32
BF16 = mybir.dt.bfloat16
I16 = mybir.dt.int16

S = 2048
E = 1024
HL = 4        # heads per core
D = 64
QB = 512
NQB = S // QB
NKT = S // 128
NKC = E // 128

# Schraudolph exp2-in-bf16-bits: bf16(2^t) bits ~= int16((t + 127 - c) * 128)
LOG2E_128 = 128.0 * 1.4426950408889634
EXP_BIAS_128 = 128.0 * (127.0 - 0.04367)

_CACHE = {}


def _dve_kt(kt, pr):
    """Which (kt, pair) exp tiles go to the DVE int-trick (rest go to ACT)."""
    return kt % 2 == 1


def _build(repeat=1):
    nc = bacc.Bacc("TRN2", target_bir_lowering=False, debug=False, num_devices=8)

    xT_d = nc.dram_tensor("xT", [E, S], BF16, kind="ExternalInput").ap()
    wqk_d = nc.dram_tensor("wqk", [E + 1, 512], BF16, kind="ExternalInput").ap()
    wv_d = nc.dram_tensor("wv", [E + 1, HL * 65], BF16, kind="ExternalInput").ap()
    wo_d = nc.dram_tensor("wo", [256, E], BF16, kind="ExternalInput").ap()
    ones_d = nc.dram_tensor("ones", [1, S], BF16, kind="ExternalInput").ap()
    y_d = nc.dram_tensor("y", [S, E], BF16, kind="ExternalOutput").ap()
    den_d = nc.dram_tensor("den_scratch", [HL * NQB, QB], F32)

    with tile.TileContext(nc) as tc:
        with (
            tc.tile_pool(name="wpool", bufs=1) as wpool,
            tc.tile_pool(name="big", bufs=1) as big,
            tc.tile_pool(name="xpool", bufs=12) as xpool,
            tc.tile_pool(name="espool", bufs=34) as espool,
            tc.tile_pool(name="denpool", bufs=4) as denpool,
            tc.tile_pool(name="bcpool", bufs=4) as bcpool,
            tc.tile_pool(name="ypool", bufs=3) as ypool,
            tc.tile_pool(name="pp", bufs=2, space="PSUM") as pp,
            tc.tile_pool(name="po", bufs=2, space="PSUM") as po,
            tc.tile_pool(name="pm", bufs=2, space="PSUM") as pm,
        ):
          with (tc.For_i(0, repeat, 1) if repeat > 1 else nullcontext()):
            wqk_t = []
            for k in range(NKC):
                w = wpool.tile([128, 512], BF16, name=f"wqk{k}")
                nc.sync.dma_start(w[:], wqk_d[k * 128:(k + 1) * 128, :])
                wqk_t.append(w)
            wqk_b = wpool.tile([1, 512], BF16, name="wqk_b")
            nc.sync.dma_start(wqk_b[:], wqk_d[E:E + 1, :])
            wv_t = []
            for k in range(NKC):
                w = wpool.tile([128, HL * 65], BF16, name=f"wv{k}")
                nc.sync.dma_start(w[:], wv_d[k * 128:(k + 1) * 128, :])
                wv_t.append(w)
            wv_b = wpool.tile([1, HL * 65], BF16, name="wv_b")
            nc.sync.dma_start(wv_b[:], wv_d[E:E + 1, :])
            wo_t = []
            for c in range(2):
                w = wpool.tile([128, E], BF16, name=f"wo{c}")
                nc.sync.dma_start(w[:], wo_d[c * 128:(c + 1) * 128, :])
                wo_t.append(w)
            ones_r = wpool.tile([1, S], BF16, name="ones_r")
            nc.sync.dma_start(ones_r[:], ones_d[:])

            qkT = [big.tile([128, S], BF16, name=f"qkT{m}") for m in range(4)]
            vt = [big.tile([128, HL, 65], BF16, name=f"vt{st}") for st in range(NKT)]
            a_t = [big.tile([128, S], BF16, name=f"a{c}") for c in range(2)]

            # ===== S1: q/k projection (all qb) =====
            for qb in range(NQB):
                xc = []
                for k in range(NKC):
                    x = xpool.tile([128, QB], BF16, name="xc")
                    nc.sync.dma_start(x[:], xT_d[k * 128:(k + 1) * 128, qb * QB:(qb + 1) * QB])
                    xc.append(x)
                for m in range(4):
                    p = pp.tile([128, QB], F32, name="pp_t", tag="pp")
                    for k in range(NKC):
                        nc.tensor.matmul(p[:], wqk_t[k][:, m * 128:(m + 1) * 128],
                                         xc[k][:], start=(k == 0), stop=False)
                    nc.tensor.matmul(p[:], wqk_b[:, m * 128:(m + 1) * 128],
                                     ones_r[:, qb * QB:(qb + 1) * QB],
                                     start=False, stop=True)
                    nc.vector.tensor_copy(qkT[m][:, qb * QB:(qb + 1) * QB], p[:])

            def emit_scores(qb, pr):
                """A-phase: 16 kt score tiles for head pair pr, row-tiled 2-up,
                plus the exp evacuation (ACT or DVE int-trick)."""
                qT = qkT[pr]
                kT = qkT[2 + pr]
                es_list = []
                for kt in range(NKT):
                    ps = pp.tile([128, 2 * QB], F32, name="ps_t", tag="pp")
                    nc.tensor.matmul(ps[:, 0:QB],
                                     kT[0:64, kt * 128:(kt + 1) * 128],
                                     qT[0:64, qb * QB:(qb + 1) * QB],
                                     start=True, stop=True, tile_position=(0, 0))
                    nc.tensor.matmul(ps[:, QB:2 * QB],
                                     kT[64:128, kt * 128:(kt + 1) * 128],
                                     qT[64:128, qb * QB:(qb + 1) * QB],
                                     start=True, stop=True, tile_position=(64, 0))
                    es = espool.tile([128, 2 * QB], BF16, name="es")
                    if _dve_kt(kt, pr):
                        nc.vector.tensor_scalar(
                            es[:].bitcast(I16), ps[:],
                            LOG2E_128, EXP_BIAS_128,
                            mybir.AluOpType.mult, mybir.AluOpType.add)
                    else:
                        nc.scalar.activation(es[:], ps[:],
                                             mybir.ActivationFunctionType.Exp)
                    es_list.append(es)
                return es_list

            def emit_pv(qb, pr, es_list):
                """B-phase + normalize for head pair pr."""
                j0, j1 = 2 * pr, 2 * pr + 1
                po_a = po.tile([65, QB], F32, name="po_a", tag="po")
                po_b = po.tile([65, QB], F32, name="po_b", tag="po")
                for kt in range(NKT):
                    es = es_list[kt]
                    nc.tensor.matmul(po_a[:], vt[kt][:, j0, :], es[:, 0:QB],
                                     start=(kt == 0), stop=(kt == NKT - 1))
                    nc.tensor.matmul(po_b[:], vt[kt][:, j1, :], es[:, QB:2 * QB],
                                     start=(kt == 0), stop=(kt == NKT - 1))
                for j, po_t in ((j0, po_a), (j1, po_b)):
                    den_r = denpool.tile([1, QB], F32, name="den_r")
                    nc.vector.reciprocal(den_r[:], po_t[64:65, :])
                    slot = j * NQB + qb
                    nc.sync.dma_start(den_d[slot:slot + 1, :], den_r[:])
                    bc = bcpool.tile([64, QB], F32, name="bc")
                    nc.sync.dma_start(bc[:], den_d[slot:slot + 1, :].to_broadcast((64, QB)))
                    a_out = a_t[j // 2][(j % 2) * 64:(j % 2) * 64 + 64,
                                        qb * QB:(qb + 1) * QB]
                    nc.vector.tensor_mul(a_out, po_t[0:64, :], bc[:])

            def emit_p3(qb):
                for sl in range(4):
                    st = qb * 4 + sl
                    for n in range(2):
                        p = pm.tile([128, QB], F32, name="pm_t", tag="pm")
                        nc.tensor.matmul(p[:], a_t[0][:, st * 128:(st + 1) * 128],
                                         wo_t[0][:, n * QB:(n + 1) * QB],
                                         start=True, stop=False)
                        nc.tensor.matmul(p[:], a_t[1][:, st * 128:(st + 1) * 128],
                                         wo_t[1][:, n * QB:(n + 1) * QB],
                                         start=False, stop=True)
                        yt = ypool.tile([128, QB], BF16, name="yt")
                        nc.scalar.copy(yt[:], p[:])
                        nc.sync.dma_start(y_d[st * 128:(st + 1) * 128,
                                              n * QB:(n + 1) * QB], yt[:])

            # A(0,0) ahead of the v projection so exp overlaps it
            es00 = emit_scores(0, 0)

            # ===== S1b: v projection (all st) =====
            for qb in range(NQB):
                xc = []
                for k in range(NKC):
                    x = xpool.tile([128, QB], BF16, name="xc")
                    nc.sync.dma_start(x[:], xT_d[k * 128:(k + 1) * 128, qb * QB:(qb + 1) * QB])
                    xc.append(x)
                for sl in range(4):
                    st = qb * 4 + sl
                    p = pm.tile([128, HL * 65], F32, name="pm_t", tag="pm")
                    for k in range(NKC):
                        nc.tensor.matmul(p[:], xc[k][:, sl * 128:(sl + 1) * 128],
                                         wv_t[k][:], start=(k == 0), stop=False)
                    nc.tensor.matmul(p[:], ones_r[:, st * 128:(st + 1) * 128], wv_b[:],
                                     start=False, stop=True)
                    nc.vector.tensor_copy(vt[st][:, :, :].rearrange("p h d -> p (h d)"), p[:])

            # ===== S2/S3 pipeline =====
            emit_pv(0, 0, es00)
            es = emit_scores(0, 1)
            emit_pv(0, 1, es)
            emit_p3(0)
            for qb in range(1, NQB):
                for pr in range(2):
                    es = emit_scores(qb, pr)
                    emit_pv(qb, pr, es)
                emit_p3(qb)

    nc.compile()
    return nc


def _shard_inputs(query, W_qkv, b_qkv, W_out, b_out):
    scale = np.float32(1.0 / np.sqrt(D))
    query = np.asarray(query, dtype=np.float32)
    W_qkv = np.asarray(W_qkv, dtype=np.float32)
    b_qkv = np.asarray(b_qkv, dtype=np.float32)
    W_out = np.asarray(W_out, dtype=np.float32)

    W_q, W_k, W_v = W_qkv[:, :E], W_qkv[:, E:2 * E], W_qkv[:, 2 * E:]
    b_q, b_k, b_v = b_qkv[:E], b_qkv[E:2 * E], b_qkv[2 * E:]

    bf = ml_dtypes.bfloat16
    ones = np.ones((1, S), bf)
    in_maps = []
    for c in range(8):
        b = c // 4
        g = c % 4
        hsl = slice(4 * g * D, (4 * g + 4) * D)
        wqk = np.empty((E + 1, 512), np.float32)
        wqk[:E, :256] = W_q[:, hsl] * scale
        wqk[E, :256] = b_q[hsl] * scale
        wqk[:E, 256:] = W_k[:, hsl]
        wqk[E, 256:] = b_k[hsl]
        wv = np.zeros((E + 1, HL * 65), np.float32)
        for j in range(HL):
            js = slice(4 * g * D + j * D, 4 * g * D + (j + 1) * D)
            wv[:E, j * 65:j * 65 + 64] = W_v[:, js]
            wv[E, j * 65:j * 65 + 64] = b_v[js]
            wv[E, j * 65 + 64] = 1.0
        in_maps.append({
            "xT": np.ascontiguousarray(query[b].T).astype(bf),
            "wqk": wqk.astype(bf),
            "wv": wv.astype(bf),
            "wo": W_out[hsl, :].astype(bf),
            "ones": ones,
        })
    return in_maps


def kernel(query, W_qkv, b_qkv, W_out, b_out):
    if "nc" not in _CACHE:
        _CACHE["nc"] = _build()
    nc = _CACHE["nc"]
    in_maps = _shard_inputs(query, W_qkv, b_qkv, W_out, b_out)
    res = run_bass_kernel_spmd(nc, in_maps, list(range(8)))
    out = np.zeros((2, S, E), np.float32)
    for c in range(8):
        out[c // 4] += np.asarray(res.results[c]["y"]).astype(np.float32)
    out += np.asarray(b_out, dtype=np.float32)
    return out


# revision 9
# speedup vs baseline: 1.3254x; 1.3254x over previous
"""Multi-head attention (B=2, S=2048, E=1024, H=16, D=64) on 8 Trainium2 NeuronCores.

Sharding (tensor-parallel over heads x data-parallel over batch):
core c -> batch b=c//4, head-group g=c%4 (4 heads per core). Each core runs the
full per-group attention pipeline; the host sums the 4 partial out-projections
per batch element and adds b_out.

Per-core pipeline (all matmul streams bf16, PSUM accumulation fp32). The PE
executes strictly in order, so emission interleaves phases to avoid stalls:
scores for head-pair phase i are woven with PV matmuls of phase i-1, the
V-projection (first phase) and out-projection blocks (later phases), while
exp evacuation alternates between ACT (true Exp) and DVE (Schraudolph int16
exp2 trick through a bitcast AP).
"""

from contextlib import nullcontext

import numpy as np
import ml_dtypes
import concourse.bass as bass
import concourse.tile as tile
from concourse import bacc, mybir
from concourse.bass_utils import run_bass_kernel_spmd

F32 = mybir.dt.float32
BF16 = mybir.dt.bfloat16
I16 = mybir.dt.int16

S = 2048
E = 1024
HL = 4        # heads per core
D = 64
QB = 512
NQB = S // QB
NKT = S // 128
NKC = E // 128

# Schraudolph exp2-in-bf16-bits: bf16(2^t) bits ~= int16((t + 127 - c) * 128)
LOG2E_128 = 128.0 * 1.4426950408889634
EXP_BIAS_128 = 128.0 * (127.0 - 0.04367)

_CACHE = {}


def _build(repeat=1, with_bias=False, use_tilepos=False):
    nc = bacc.Bacc("TRN2", target_bir_lowering=False, debug=False, num_devices=8)

    xT_d = nc.dram_tensor("xT", [E, S], BF16, kind="ExternalInput").ap()
    wqk_d = nc.dram_tensor("wqk", [E + 1, 512], BF16, kind="ExternalInput").ap()
    wv_d = nc.dram_tensor("wv", [E + 1, HL * 65], BF16, kind="ExternalInput").ap()
    wo_d = nc.dram_tensor("wo", [256, E], BF16, kind="ExternalInput").ap()
    ones_d = nc.dram_tensor("ones", [1, S], BF16, kind="ExternalInput").ap()
    y_d = nc.dram_tensor("y", [S, E], BF16, kind="ExternalOutput").ap()
    den_d = nc.dram_tensor("den_scratch", [HL * NQB, QB], F32)

    with tile.TileContext(nc) as tc:
        with (
            tc.tile_pool(name="wpool", bufs=1) as wpool,
            tc.tile_pool(name="xres", bufs=1) as xres,
            tc.tile_pool(name="big", bufs=1) as big,
            tc.tile_pool(name="espool", bufs=34) as espool,
            tc.tile_pool(name="denpool", bufs=4) as denpool,
            tc.tile_pool(name="bcpool", bufs=4) as bcpool,
            tc.tile_pool(name="ypool", bufs=3) as ypool,
            tc.tile_pool(name="pp", bufs=2, space="PSUM") as pp,
            tc.tile_pool(name="po", bufs=3, space="PSUM") as po,
            tc.tile_pool(name="pm", bufs=1, space="PSUM") as pm,
        ):
          with (tc.For_i(0, repeat, 1) if repeat > 1 else nullcontext()):
            # --- weights + x (resident) ---
            wqk_t = []
            for k in range(NKC):
                w = wpool.tile([128, 512], BF16, name=f"wqk{k}")
                nc.sync.dma_start(w[:], wqk_d[k * 128:(k + 1) * 128, :])
                wqk_t.append(w)
            wv_t = []
            for k in range(NKC):
                w = wpool.tile([128, HL * 65], BF16, name=f"wv{k}")
                nc.sync.dma_start(w[:], wv_d[k * 128:(k + 1) * 128, :])
                wv_t.append(w)
            wo_t = []
            for c in range(2):
                w = wpool.tile([128, E], BF16, name=f"wo{c}")
                nc.sync.dma_start(w[:], wo_d[c * 128:(c + 1) * 128, :])
                wo_t.append(w)
            if with_bias:
                wqk_b = wpool.tile([1, 512], BF16, name="wqk_b")
                nc.sync.dma_start(wqk_b[:], wqk_d[E:E + 1, :])
                wv_b = wpool.tile([1, HL * 65], BF16, name="wv_b")
                nc.sync.dma_start(wv_b[:], wv_d[E:E + 1, :])
                ones_r = wpool.tile([1, S], BF16, name="ones_r")
                nc.sync.dma_start(ones_r[:], ones_d[:])
            xr = []
            for k in range(NKC):
                x = xres.tile([128, S], BF16, name=f"xr{k}")
                nc.sync.dma_start(x[:], xT_d[k * 128:(k + 1) * 128, :])
                xr.append(x)

            qkT = [big.tile([128, S], BF16, name=f"qkT{m}") for m in range(4)]
            vt = [big.tile([128, HL, 65], BF16, name=f"vt{st}") for st in range(NKT)]
            a_t = [big.tile([128, S], BF16, name=f"a{c}") for c in range(2)]

            # --- stage 1: q/k projection ---
            for qb in range(NQB):
                qsl = slice(qb * QB, (qb + 1) * QB)
                for m in range(4):
                    p = pp.tile([128, QB], F32, name="pp_t", tag="pp")
                    for k in range(NKC):
                        nc.tensor.matmul(p[:], wqk_t[k][:, m * 128:(m + 1) * 128],
                                         xr[k][:, qsl], start=(k == 0),
                                         stop=(k == NKC - 1) and not with_bias)
                    if with_bias:
                        nc.tensor.matmul(p[:], wqk_b[:, m * 128:(m + 1) * 128],
                                         ones_r[:, qsl], start=False, stop=True)
                    nc.vector.tensor_copy(qkT[m][:, qsl], p[:])

            # --- emission helpers -------------------------------------------
            def emit_score(qb, pr, kt):
                """One [128,1024] score tile (head pair pr) + exp evacuation."""
                qT = qkT[pr]
                kT = qkT[2 + pr]
                ksl = slice(kt * 128, (kt + 1) * 128)
                qsl = slice(qb * QB, (qb + 1) * QB)
                ps = pp.tile([128, 2 * QB], F32, name="ps_t", tag="pp")
                tp = dict(tile_position=(0, 0)) if use_tilepos else {}
                nc.tensor.matmul(ps[:, 0:QB], kT[0:64, ksl], qT[0:64, qsl],
                                 start=True, stop=True, **tp)
                tp = dict(tile_position=(64, 0)) if use_tilepos else {}
                nc.tensor.matmul(ps[:, QB:2 * QB], kT[64:128, ksl], qT[64:128, qsl],
                                 start=True, stop=True, **tp)
                es = espool.tile([128, 2 * QB], BF16, name="es")
                if kt % 2 == 1:
                    nc.vector.tensor_scalar(
                        es[:].bitcast(I16), ps[:], LOG2E_128, EXP_BIAS_128,
                        mybir.AluOpType.mult, mybir.AluOpType.add)
                else:
                    nc.scalar.activation(es[:], ps[:],
                                         mybir.ActivationFunctionType.Exp)
                return es

            def emit_pv_group(ph, g):
                """The two PV matmuls for k-tile g of phase ph."""
                nc.tensor.matmul(ph["po_a"][:], vt[g][:, ph["j0"], :],
                                 ph["es"][g][:, 0:QB],
                                 start=(g == 0), stop=(g == NKT - 1))
                nc.tensor.matmul(ph["po_b"][:], vt[g][:, ph["j1"], :],
                                 ph["es"][g][:, QB:2 * QB],
                                 start=(g == 0), stop=(g == NKT - 1))

            def emit_recip(ph):
                for j, po_t in ((ph["j0"], ph["po_a"]), (ph["j1"], ph["po_b"])):
                    den_r = denpool.tile([1, QB], F32, name="den_r")
                    nc.vector.reciprocal(den_r[:], po_t[64:65, :])
                    slot = j * NQB + ph["qb"]
                    nc.sync.dma_start(den_d[slot:slot + 1, :], den_r[:])
                    bc = bcpool.tile([64, QB], F32, name="bc")
                    nc.sync.dma_start(bc[:], den_d[slot:slot + 1, :]
                                      .to_broadcast((64, QB)))
                    ph.setdefault("bc", []).append(bc)

            def emit_amul(ph):
                qsl = slice(ph["qb"] * QB, (ph["qb"] + 1) * QB)
                for i, (j, po_t) in enumerate(((ph["j0"], ph["po_a"]),
                                              (ph["j1"], ph["po_b"]))):
                    a_out = a_t[j // 2][(j % 2) * 64:(j % 2) * 64 + 64, qsl]
                    nc.vector.tensor_mul(a_out, po_t[0:64, :], ph["bc"][i][:])

            def emit_vt_block(st):
                """V projection for one 128-row S block (woven into phase 0)."""
                qsl128 = slice(st * 128, (st + 1) * 128)
                p = pm.tile([128, HL * 65], F32, name="pm_t", tag="pm")
                for k in range(NKC):
                    nc.tensor.matmul(p[:], xr[k][:, qsl128],
                                     wv_t[k][:], start=(k == 0),
                                     stop=(k == NKC - 1) and not with_bias)
                if with_bias:
                    nc.tensor.matmul(p[:], ones_r[:, qsl128], wv_b[:],
                                     start=False, stop=True)
                nc.vector.tensor_copy(
                    vt[st][:, :, :].rearrange("p h d -> p (h d)"), p[:])
                if not with_bias:
                    # the unit column that accumulates the softmax denominator
                    nc.gpsimd.memset(vt[st][:, :, 64:65], 1.0)

            def emit_p3_block(qb, blk):
                """One (st, n) out-projection block: 2 MMs + copy + store."""
                st = qb * 4 + blk // 2
                n = blk % 2
                p = pm.tile([128, QB], F32, name="pm_t", tag="pm")
                nc.tensor.matmul(p[:], a_t[0][:, st * 128:(st + 1) * 128],
                                 wo_t[0][:, n * QB:(n + 1) * QB],
                                 start=True, stop=False)
                nc.tensor.matmul(p[:], a_t[1][:, st * 128:(st + 1) * 128],
                                 wo_t[1][:, n * QB:(n + 1) * QB],
                                 start=False, stop=True)
                yt = ypool.tile([128, QB], BF16, name="yt")
                nc.scalar.copy(yt[:], p[:])
                nc.sync.dma_start(y_d[st * 128:(st + 1) * 128,
                                      n * QB:(n + 1) * QB], yt[:])

            # --- stage 2: interleaved phase machine -------------------------
            prev = None
            for idx, (qb, pr) in enumerate((q, p) for q in range(NQB)
                                           for p in range(2)):
                cur = {"qb": qb, "pr": pr, "j0": 2 * pr, "j1": 2 * pr + 1,
                       "es": [],
                       "po_a": po.tile([65, QB], F32, name="po_a", tag="po"),
                       "po_b": po.tile([65, QB], F32, name="po_b", tag="po")}
                # out-projection of batch row-block finished two phases ago
                p3_qb = qb - 1 if (pr == 1 and qb >= 1) else None
                for kt in range(NKT):
                    cur["es"].append(emit_score(qb, pr, kt))
                    if prev is not None and kt < 8:
                        emit_pv_group(prev, 2 * kt)
                        emit_pv_group(prev, 2 * kt + 1)
                        if kt == 7:
                            emit_recip(prev)
                    if idx == 0:
                        emit_vt_block(kt)
                    if prev is not None and kt == 12:
                        emit_amul(prev)
                    if p3_qb is not None and kt % 2 == 1:
                        emit_p3_block(p3_qb, kt // 2)
                prev = cur
            # tail: last phase PV + normalize + last out-projection
            for g in range(NKT):
                emit_pv_group(prev, g)
            emit_recip(prev)
            emit_amul(prev)
            for blk in range(8):
                emit_p3_block(NQB - 1, blk)

    nc.compile()
    return nc


def _shard_inputs(query, W_qkv, b_qkv, W_out, b_out):
    scale = np.float32(1.0 / np.sqrt(D))
    query = np.asarray(query, dtype=np.float32)
    W_qkv = np.asarray(W_qkv, dtype=np.float32)
    b_qkv = np.asarray(b_qkv, dtype=np.float32)
    W_out = np.asarray(W_out, dtype=np.float32)

    W_q, W_k, W_v = W_qkv[:, :E], W_qkv[:, E:2 * E], W_qkv[:, 2 * E:]
    b_q, b_k, b_v = b_qkv[:E], b_qkv[E:2 * E], b_qkv[2 * E:]

    bf = ml_dtypes.bfloat16
    ones = np.ones((1, S), bf)
    in_maps = []
    for c in range(8):
        b = c // 4
        g = c % 4
        hsl = slice(4 * g * D, (4 * g + 4) * D)
        wqk = np.empty((E + 1, 512), np.float32)
        wqk[:E, :256] = W_q[:, hsl] * scale
        wqk[E, :256] = b_q[hsl] * scale
        wqk[:E, 256:] = W_k[:, hsl]
        wqk[E, 256:] = b_k[hsl]
        wv = np.zeros((E + 1, HL * 65), np.float32)
        for j in range(HL):
            js = slice(4 * g * D + j * D, 4 * g * D + (j + 1) * D)
            wv[:E, j * 65:j * 65 + 64] = W_v[:, js]
            wv[E, j * 65:j * 65 + 64] = b_v[js]
            wv[E, j * 65 + 64] = 1.0
        in_maps.append({
            "xT": np.ascontiguousarray(query[b].T).astype(bf),
            "wqk": wqk.astype(bf),
            "wv": wv.astype(bf),
            "wo": W_out[hsl, :].astype(bf),
            "ones": ones,
        })
    return in_maps


def kernel(query, W_qkv, b_qkv, W_out, b_out):
    with_bias = bool(np.any(np.asarray(b_qkv)))
    key = ("nc", with_bias)
    if key not in _CACHE:
        _CACHE[key] = _build(with_bias=with_bias)
    nc = _CACHE[key]
    in_maps = _shard_inputs(query, W_qkv, b_qkv, W_out, b_out)
    res = run_bass_kernel_spmd(nc, in_maps, list(range(8)))
    out = np.zeros((2, S, E), np.float32)
    for c in range(8):
        out[c // 4] += np.asarray(res.results[c]["y"]).astype(np.float32)
    out += np.asarray(b_out, dtype=np.float32)
    return out
